# revision 9
# baseline (speedup 1.0000x reference)
"""Trainium2 Bass kernel for nn_SimpleDecoder (greedy LSTM decoder).

Structure of the reference: a 128-step greedy decode loop. The carry init
(h=c=0, tok=0) is identical for every batch row and `text` is never read, so
all 32 batch rows compute the same trajectory — we compute one trajectory
(replicated across 32 PE lanes) and replicate the output on the host.

Distribution: vocab-parallel over the 8 cores (W_fc/b_fc sharded by 4000
vocab rows). Each step: every core computes the LSTM cell redundantly
(fp32r matmuls), its fc shard (bf16, 4x column-tiled PE), a local argmax
(DVE max8/max_index), then an 8-core AllGather of (max, argmax) candidates
picks the global token, which is gathered from the embedding table by
indirect DMA to feed the next step.
"""
import sys

sys.path.insert(0, "/opt/trn_rl_repo")

import numpy as np
import ml_dtypes

import concourse.bacc as bacc
import concourse.mybir as mybir
import concourse.tile as tile
from concourse.bass import IndirectOffsetOnAxis

F32 = mybir.dt.float32
F32R = mybir.dt.float32r
BF16 = mybir.dt.bfloat16
I32 = mybir.dt.int32
U32 = mybir.dt.uint32

NCORES = 8
VOCAB, EMBED, HIDDEN, BATCH, T = 32000, 256, 512, 32, 128
VS = VOCAB // NCORES  # 4000 per core
BIG = 65536.0

_SIG = mybir.ActivationFunctionType.Sigmoid
_TANH = mybir.ActivationFunctionType.Tanh


def _round_f32r(a):
    u = np.ascontiguousarray(a, np.float32).view(np.uint32)
    r = (u + 0x7FF + ((u >> 12) & 1)) & 0xFFFFF000
    return r.view(np.float32).reshape(np.asarray(a).shape)


# f32r blob [128, RE] (matmul weights; runtime truncates low 12 bits on upload)
class R:
    wih = 0                # [128, 2*2048]   wih[p, 2048k+j] = W_ih[j, 128k+p]
    whh = 4096             # [128, 4*2048]   whh[p, 2048k+j] = W_hh[j, 128k+p]
    bg = 12288             # [128, 2048]     row0 = b_ih + b_hh
    ones = 14336           # [128, 32]       1.0
    end = 14368


# f32 blob [128, FE] (exact data)
class F:
    ident = 0              # [128, 128] identity
    grp = 128              # [128, 1]   4000*core + 500*(p//32)
    end = 129


# bf16 blob [128, BE]
class B:
    wfc = 0                # [128, 4*4000]  wfc[p, 4000k+n] = W_fc[shard0+n, 128k+p]
    bfc = 16000            # [128, 4000]    row0 = bf16(b), row1 = bf16(b - hi)
    ones = 20000           # [128, 32]      1.0 (rows 0-1)
    end = 20032


def build(nsteps):
    nc = bacc.Bacc(None, target_bir_lowering=False)

    blob_r_in = nc.dram_tensor("blob_r", [128, R.end], F32R, kind="ExternalInput")
    blob_f_in = nc.dram_tensor("blob_f", [128, F.end], F32, kind="ExternalInput")
    blob_b_in = nc.dram_tensor("blob_b", [128, B.end], BF16, kind="ExternalInput")
    embed_in = nc.dram_tensor("embed", [VOCAB, EMBED], F32, kind="ExternalInput")
    out_t = nc.dram_tensor("logits_out", [nsteps, VS], F32, kind="ExternalOutput")

    with tile.TileContext(nc) as tc:
        with (
            tc.tile_pool(name="w", bufs=1) as wp,      # persistent weights/state
            tc.tile_pool(name="s", bufs=2) as sp,      # per-step tiles
            tc.tile_pool(name="ps", bufs=1, space="PSUM") as pp,
            tc.tile_pool(name="psfc", bufs=2, space="PSUM") as pfc,
            tc.tile_pool(name="dram", bufs=2, space="DRAM") as dp,
        ):
            br = wp.tile([128, R.end], F32R, tag="br")
            nc.sync.dma_start(br[:], blob_r_in[:, :])
            bf = wp.tile([128, F.end], F32, tag="bf")
            nc.sync.dma_start(bf[:], blob_f_in[:, :])
            bb = wp.tile([128, B.end], BF16, tag="bb")
            nc.sync.dma_start(bb[:], blob_b_in[:, :])

            # persistent state
            c_sb = wp.tile([32, HIDDEN], F32, tag="c_sb")
            nc.vector.memset(c_sb[:], 0.0)
            hT_r = wp.tile([128, 128], F32R, tag="hT_r")
            nc.vector.memset(hT_r[:].bitcast(F32), 0.0)
            hT_b = wp.tile([128, 128], BF16, tag="hT_b")
            nc.vector.memset(hT_b[:], 0.0)
            toki = wp.tile([32, 1], I32, tag="toki")
            nc.vector.memset(toki[:], 0)

            ident32 = bf[0:32, F.ident : F.ident + 32]
            ones_r = br[0:1, R.ones : R.ones + 32]
            ones_b2 = bb[0:2, B.ones : B.ones + 32]
            grp = bf[:, F.grp : F.grp + 1]

            for t in range(nsteps):
                # ---- gates bias + h-part first (independent of tok_t, so the
                # in-order PE can run them while the token exchange is in flight)
                gp = pp.tile([128, 2048], F32, tag="gp")
                for n in range(4):
                    cs = 512 * n
                    nc.tensor.matmul(
                        gp[0:32, cs : cs + 512], ones_r,
                        br[0:1, R.bg + cs : R.bg + cs + 512],
                        start=True, stop=False, skip_group_check=True,
                    )
                    for k in range(4):
                        nc.tensor.matmul(
                            gp[0:32, cs : cs + 512], hT_r[:, 32 * k : 32 * k + 32],
                            br[:, R.whh + 2048 * k + cs : R.whh + 2048 * k + cs + 512],
                            start=False, stop=False, skip_group_check=True,
                        )

                # ---- fc bias matmuls early: tok-independent PE work that keeps
                # the PE busy (HAM-warm) through the token-exchange window
                fps = []
                for half in range(2):
                    fp = pfc.tile([128, 512], F32, tag="fp")
                    for j in range(4):
                        vcol = 2000 * half + 500 * j
                        nc.tensor.matmul(
                            fp[32 * j : 32 * j + 32, 0:500], ones_b2,
                            bb[0:2, B.bfc + vcol : B.bfc + vcol + 500],
                            start=True, stop=False,
                            tile_position=(0, 32 * j), skip_group_check=True,
                        )
                    fps.append(fp)

                # ---- x = embed[tok] ----
                xg = sp.tile([32, EMBED], F32, tag="xg")
                nc.gpsimd.indirect_dma_start(
                    out=xg[:], out_offset=None, in_=embed_in[:, :],
                    in_offset=IndirectOffsetOnAxis(ap=toki[:, 0:1], axis=0),
                )
                # x.T via PE transpose -> bank 1 of tp_ps (bank 0 holds h.T)
                tp_ps = pp.tile([128, 1024], F32, tag="tp_ps")
                for k in range(2):
                    nc.tensor.transpose(
                        out=tp_ps[:, 512 + 32 * k : 544 + 32 * k],
                        in_=xg[0:32, 128 * k : 128 * k + 128],
                        identity=ident32,
                    )
                xT = sp.tile([128, 64], F32R, tag="xT")
                nc.vector.tensor_copy(xT[:], tp_ps[:, 512:576])

                # ---- gates x-part ----
                for n in range(4):
                    cs = 512 * n
                    for k in range(2):
                        nc.tensor.matmul(
                            gp[0:32, cs : cs + 512], xT[:, 32 * k : 32 * k + 32],
                            br[:, R.wih + 2048 * k + cs : R.wih + 2048 * k + cs + 512],
                            start=False, stop=(k == 1), skip_group_check=True,
                        )

                # ---- HAM filler: junk matmuls so the PE stays warm during the
                # ~5us activation/cell window (idle >3.4us would re-throttle)
                for jj in range(10):
                    nc.tensor.matmul(
                        tp_ps[0:32, 576:1024],
                        hT_r[:, 0:32],
                        br[:, R.whh + 448 * (jj % 4) : R.whh + 448 * (jj % 4) + 448],
                        start=(jj == 0), stop=(jj == 9), skip_group_check=True,
                    )

                # ---- activations: i,f sigmoid | g tanh | o sigmoid ----
                gsb = sp.tile([32, 2048], F32, tag="gsb")
                nc.scalar.activation(gsb[:, 0:1024], gp[0:32, 0:1024], _SIG)
                nc.scalar.activation(gsb[:, 1024:1536], gp[0:32, 1024:1536], _TANH)
                nc.scalar.activation(gsb[:, 1536:2048], gp[0:32, 1536:2048], _SIG)

                # ---- cell ----
                t_fc = sp.tile([32, HIDDEN], F32, tag="t_fc")
                nc.gpsimd.tensor_tensor(
                    out=t_fc[:], in0=gsb[:, 512:1024], in1=c_sb[:],
                    op=mybir.AluOpType.mult,
                )
                t_ig = sp.tile([32, HIDDEN], F32, tag="t_ig")
                nc.vector.tensor_tensor(
                    out=t_ig[:], in0=gsb[:, 0:512], in1=gsb[:, 1024:1536],
                    op=mybir.AluOpType.mult,
                )
                nc.vector.tensor_add(c_sb[:], t_fc[:], t_ig[:])
                tnh = sp.tile([32, HIDDEN], F32, tag="tnh")
                nc.scalar.activation(tnh[:], c_sb[:], _TANH)
                h_sb = sp.tile([32, HIDDEN], F32, tag="h_sb")
                nc.vector.tensor_tensor(
                    out=h_sb[:], in0=gsb[:, 1536:2048], in1=tnh[:],
                    op=mybir.AluOpType.mult,
                )

                # ---- h.T -> f32r + bf16 stationaries ----
                for k in range(4):
                    nc.tensor.transpose(
                        out=tp_ps[:, 32 * k : 32 * k + 32],
                        in_=h_sb[0:32, 128 * k : 128 * k + 128],
                        identity=ident32,
                    )
                nc.vector.tensor_copy(hT_r[:], tp_ps[:, 0:128])
                nc.vector.tensor_copy(hT_b[:], tp_ps[:, 0:128])

                # ---- fc shard: 2 psum tiles x 4 col groups x 500 ----
                mx = []
                mi = []
                for half in range(2):
                    fp = fps[half]
                    for j in range(4):
                        vcol = 2000 * half + 500 * j
                        for k in range(4):
                            nc.tensor.matmul(
                                fp[32 * j : 32 * j + 32, 0:500],
                                hT_b[:, 32 * k : 32 * k + 32],
                                bb[:, B.wfc + 4000 * k + vcol : B.wfc + 4000 * k + vcol + 500],
                                start=False, stop=(k == 3),
                                tile_position=(0, 32 * j), skip_group_check=True,
                            )
                    # argmax straight from PSUM (skips waiting on the copy)
                    m8 = sp.tile([128, 8], F32, tag=f"mx{half}")
                    nc.vector.max(out=m8[:], in_=fp[:, 0:500])
                    i8 = sp.tile([128, 8], U32, tag=f"mi{half}")
                    nc.vector.max_index(out=i8[:], in_max=m8[:], in_values=fp[:, 0:500])
                    mx.append(m8)
                    mi.append(i8)
                    # output copy + DMA, off the critical path
                    lsb = sp.tile([128, 500], F32, tag=f"lsb{half}")
                    nc.scalar.copy(lsb[:], fp[:, 0:500])
                    nc.sync.dma_start(
                        out_t[t : t + 1, 2000 * half : 2000 * half + 2000]
                        .rearrange("a (j n) -> (a j) n", n=500),
                        lsb[0:128:32, 0:500],
                    )

                # consume the junk psum so nothing DCEs the warm-keeper matmuls
                junk_sb = sp.tile([32, 1], F32, tag="junk_sb")
                nc.scalar.copy(junk_sb[:], tp_ps[0:32, 576:577])

                # ---- merge 2 halves -> per-group candidate (val, globidx) ----
                ig0 = sp.tile([128, 1], F32, tag="ig0")
                nc.vector.tensor_add(ig0[:], mi[0][:, 0:1], grp)
                ig1 = sp.tile([128, 1], F32, tag="ig1")
                nc.vector.scalar_tensor_tensor(
                    out=ig1[:], in0=mi[1][:, 0:1], scalar=2000.0, in1=grp,
                    op0=mybir.AluOpType.add, op1=mybir.AluOpType.add,
                )
                m01 = sp.tile([128, 1], I32, tag="m01")  # select mask must be int dtype
                nc.vector.tensor_tensor(
                    out=m01[:], in0=mx[1][:, 0:1], in1=mx[0][:, 0:1],
                    op=mybir.AluOpType.is_gt,
                )
                vi = sp.tile([128, 2], F32, tag="vi")
                nc.vector.tensor_tensor(
                    out=vi[:, 0:1], in0=mx[0][:, 0:1], in1=mx[1][:, 0:1],
                    op=mybir.AluOpType.max,
                )
                nc.vector.select(vi[:, 1:2], m01[:], ig1[:], ig0[:])

                # ---- candidates [v0..v3, i0..i3] -> AllGather ----
                cand8 = sp.tile([1, 8], F32, tag="cand8")
                nc.sync.dma_start(cand8[0:1, 0:4], vi[0:128:32, 0:1])
                nc.sync.dma_start(cand8[0:1, 4:8], vi[0:128:32, 1:2])
                ag_i = dp.tile([1, 8], F32, tag="ag_i")
                ag_o = dp.tile([8, 8], F32, tag="ag_o")
                nc.gpsimd.dma_start(ag_i[:], cand8[:])
                nc.gpsimd.collective_compute(
                    "AllGather", mybir.AluOpType.bypass,
                    ins=[ag_i[:].opt()], outs=[ag_o[:].opt()],
                    replica_groups=[list(range(NCORES))],
                )
                agsb = sp.tile([1, 64], F32, tag="agsb")
                nc.sync.dma_start(agsb[0:1, 0:64], ag_o[:, :])

                # ---- global argmax over 32 (v,i) pairs ----
                ag4 = agsb[0:1, 0:64].rearrange("p (c h n) -> p c h n", h=2, n=4)
                vals, idxs = ag4[:, :, 0:1, :], ag4[:, :, 1:2, :]
                vmax = sp.tile([1, 1], F32, tag="vmax")
                nc.vector.tensor_reduce(
                    vmax[:], vals, axis=mybir.AxisListType.XYZ, op=mybir.AluOpType.max
                )
                msk = sp.tile([1, 32], F32, tag="msk")
                msk4 = msk[0:1, 0:32].rearrange("p (c h n) -> p c h n", h=1, n=4)
                nc.vector.tensor_scalar(
                    out=msk4, in0=vals, scalar1=vmax[0:1, 0:1], scalar2=None,
                    op0=mybir.AluOpType.is_equal,
                )
                xm = sp.tile([1, 32], F32, tag="xm")
                xm4 = xm[0:1, 0:32].rearrange("p (c h n) -> p c h n", h=1, n=4)
                nc.vector.scalar_tensor_tensor(
                    out=xm4, in0=msk4, scalar=-BIG, in1=idxs,
                    op0=mybir.AluOpType.mult, op1=mybir.AluOpType.add,
                )
                tokm = sp.tile([1, 1], F32, tag="tokm")
                nc.vector.tensor_reduce(
                    tokm[:], xm[:], axis=mybir.AxisListType.X, op=mybir.AluOpType.min
                )
                tokf = sp.tile([1, 1], I32, tag="tokf")
                nc.vector.tensor_scalar(
                    out=tokf[:], in0=tokm[:], scalar1=BIG, scalar2=None,
                    op0=mybir.AluOpType.add,
                )
                if t < nsteps - 1:
                    nc.sync.dma_start(toki[:, 0:1], tokf[0:1, 0:1].to_broadcast([1, 32]))

    nc.finalize()
    return nc


def _prep_core_inputs(inputs, core):
    W_ih = np.asarray(inputs["W_ih"], np.float32)
    W_hh = np.asarray(inputs["W_hh"], np.float32)
    b = (np.asarray(inputs["b_ih"], np.float32) + np.asarray(inputs["b_hh"], np.float32))
    W_fc = np.asarray(inputs["W_fc"], np.float32)
    b_fc = np.asarray(inputs["b_fc"], np.float32)
    embed = np.ascontiguousarray(np.asarray(inputs["embed"], np.float32))

    blob_r = np.zeros((128, R.end), np.float32)
    for k in range(2):
        blob_r[:, R.wih + 2048 * k : R.wih + 2048 * (k + 1)] = W_ih[:, 128 * k : 128 * k + 128].T
    for k in range(4):
        blob_r[:, R.whh + 2048 * k : R.whh + 2048 * (k + 1)] = W_hh[:, 128 * k : 128 * k + 128].T
    blob_r[0, R.bg : R.bg + 2048] = b
    blob_r[:, R.ones : R.ones + 32] = 1.0
    blob_r = _round_f32r(blob_r)

    blob_f = np.zeros((128, F.end), np.float32)
    blob_f[:, F.ident : F.ident + 128] = np.eye(128, dtype=np.float32)
    blob_f[:, F.grp] = 4000.0 * core + 500.0 * (np.arange(128) // 32)

    v0 = VS * core
    Ws = W_fc[v0 : v0 + VS]        # [4000, 512]
    bs = b_fc[v0 : v0 + VS]        # [4000]
    blob_b = np.zeros((128, B.end), ml_dtypes.bfloat16)
    for k in range(4):
        blob_b[:, B.wfc + 4000 * k : B.wfc + 4000 * (k + 1)] = (
            Ws[:, 128 * k : 128 * k + 128].T.astype(ml_dtypes.bfloat16)
        )
    bh = bs.astype(ml_dtypes.bfloat16)
    bl = (bs - bh.astype(np.float32)).astype(ml_dtypes.bfloat16)
    blob_b[0, B.bfc : B.bfc + VS] = bh
    blob_b[1, B.bfc : B.bfc + VS] = bl
    blob_b[0:2, B.ones : B.ones + 32] = 1.0

    return {
        "blob_r": blob_r,
        "blob_f": blob_f,
        "blob_b": blob_b,
        "embed": embed,
    }


_CACHED = {}


def run(inputs, nsteps=T, trace=False):
    from concourse.bass_utils import run_bass_kernel_spmd

    key = nsteps
    if key not in _CACHED:
        _CACHED[key] = build(nsteps)
    nc = _CACHED[key]
    in_maps = [_prep_core_inputs(inputs, c) for c in range(NCORES)]
    res = run_bass_kernel_spmd(nc, in_maps, core_ids=list(range(NCORES)), trace=trace)
    shards = [res.results[c]["logits_out"] for c in range(NCORES)]
    logits_tv = np.concatenate(shards, axis=1)  # [nsteps, VOCAB]
    return logits_tv, res


def kernel(**inputs) -> np.ndarray:
    logits_tv, _ = run(inputs, T)
    return np.broadcast_to(logits_tv[None], (BATCH, T, VOCAB)).copy()
